# revision 7
# baseline (speedup 1.0000x reference)
"""ChannelDisassembly Trainium kernel.

Splits "outlier" channels (per-channel max|x| > 8) into T = ceil(max/8)
identical copies scaled by 1/T.  The replication plan is computed host-side
from the global per-channel maxima (mirrors the reference's host sync), then
a Bass/Tile kernel applies the gather+scale, data-parallel over the batch
dim across 8 NeuronCores.

Per core: read its [B/8, C, H*W] batch shard, scale each outlier channel by
1/T on DVE, and DMA each scaled channel back out T times (single DMA per
channel tile using a broadcast source AP).
"""

import numpy as np

THRESHOLD = 8.0
B, C, H, W = 16, 512, 56, 56
HW = H * W
N_CORES = 8
BSH = B // N_CORES  # batches per core
P = 128  # SBUF partitions

PROFILE = False  # test harness sets True to collect exec_time_ns
STORE_BROADCAST = False  # one broadcast-src store vs T strided stores
LAST_EXEC_NS = None
LAST_RESULTS = None

_build_cache = {}


def _plan(x):
    max_vals = np.max(np.abs(x), axis=(0, 2, 3))
    outlier = np.nonzero(max_vals > THRESHOLD)[0]
    T = np.ceil(max_vals[outlier] / THRESHOLD).astype(np.int64)
    return outlier, T


def _tile_runs(outlier, T, starts, i, n_out):
    """Contiguous-source load runs and equal-T store runs for channel tile i."""
    lo, hi = i * P, min((i + 1) * P, n_out)
    chans = outlier[lo:hi]
    loads = []  # (dst_partition, src_channel, run_len)
    rs = 0
    for k in range(1, len(chans) + 1):
        if k == len(chans) or chans[k] != chans[k - 1] + 1:
            loads.append((rs, int(chans[rs]), k - rs))
            rs = k
    stores = []  # (partition0, run_len, T, out_channel_start)
    ts, st = T[lo:hi], starts[lo:hi]
    rs = 0
    for k in range(1, len(chans) + 1):
        if k == len(chans) or ts[k] != ts[rs]:
            stores.append((rs, k - rs, int(ts[rs]), int(st[rs])))
            rs = k
    return loads, stores, hi - lo


def _build(outlier, T):
    import concourse.bacc as bacc
    import concourse.mybir as mybir
    import concourse.tile as tile

    f32 = mybir.dt.float32
    n_out = len(outlier)
    n_tiles = (n_out + P - 1) // P
    starts = np.concatenate([[0], np.cumsum(T)[:-1]]).astype(np.int64)
    R = int(T.sum())

    nc = bacc.Bacc(None)
    x_d = nc.declare_dram_parameter("x", [BSH * C, HW], f32, isOutput=False)
    s_d = nc.declare_dram_parameter("s", [P, n_tiles], f32, isOutput=False)
    y_d = nc.declare_dram_parameter("y", [BSH * R, HW], f32, isOutput=True)

    with tile.TileContext(nc) as tc:
        with (
            tc.tile_pool(name="sc", bufs=1) as scp,
            tc.tile_pool(name="io", bufs=6) as iop,
        ):
            sc = scp.tile([P, n_tiles], f32)
            nc.sync.dma_start(sc[:], s_d[:])
            for b in range(BSH):
                for i in range(n_tiles):
                    loads, stores, p = _tile_runs(outlier, T, starts, i, n_out)
                    t = iop.tile([P, HW], f32)
                    for dp, src_c, ln in loads:
                        nc.sync.dma_start(
                            t[dp : dp + ln, :],
                            x_d[b * C + src_c : b * C + src_c + ln, :],
                        )
                    nc.vector.tensor_scalar_mul(t[:p, :], t[:p, :], sc[:p, i : i + 1])
                    for p0, ln, tt, os_ in stores:
                        grp = y_d[
                            b * R + os_ : b * R + os_ + ln * tt, :
                        ].rearrange("(l t) f -> l t f", t=tt)
                        if STORE_BROADCAST:
                            src = t[p0 : p0 + ln, :].unsqueeze(1).broadcast_to(
                                [ln, tt, HW]
                            )
                            nc.scalar.dma_start(grp, src)
                        else:
                            for trep in range(tt):
                                nc.scalar.dma_start(
                                    grp[:, trep, :], t[p0 : p0 + ln, :]
                                )
    nc.finalize()  # Bacc: split multi-wait instructions, allocate registers
    return nc


def kernel(x):
    global LAST_EXEC_NS, LAST_RESULTS
    from concourse.bass_utils import run_bass_kernel_spmd

    x = np.ascontiguousarray(np.asarray(x), dtype=np.float32)
    assert x.shape == (B, C, H, W), x.shape

    outlier, T = _plan(x)
    R = int(T.sum())
    if R == 0:
        return (
            np.zeros((B, 0, H, W), np.float32),
            outlier.astype(np.int32),
        )

    key = (outlier.tobytes(), T.tobytes(), STORE_BROADCAST)
    if key not in _build_cache:
        _build_cache[key] = _build(outlier, T)
    nc = _build_cache[key]

    n_out = len(outlier)
    n_tiles = (n_out + P - 1) // P
    inv_T = (1.0 / T).astype(np.float32)
    svec = np.ones((P, n_tiles), np.float32)
    for j in range(n_out):
        svec[j % P, j // P] = inv_T[j]

    xs = x.reshape(B, C, HW)
    in_maps = [
        {"x": np.ascontiguousarray(xs[c * BSH : (c + 1) * BSH].reshape(BSH * C, HW)),
         "s": svec}
        for c in range(N_CORES)
    ]
    res = run_bass_kernel_spmd(
        nc, in_maps, core_ids=list(range(N_CORES)), trace=PROFILE
    )
    LAST_EXEC_NS = res.exec_time_ns
    LAST_RESULTS = res
    out = np.concatenate(
        [r["y"].reshape(BSH, R, H, W) for r in res.results], axis=0
    )
    return out, outlier.astype(np.int32)


# revision 22
# speedup vs baseline: 1.2862x; 1.2862x over previous
"""ChannelDisassembly Trainium kernel.

Splits "outlier" channels (per-channel max|x| > 8) into T = ceil(max/8)
identical copies scaled by 1/T.  The replication plan is computed host-side
from the global per-channel maxima (mirrors the reference's host sync), then
a Bass/Tile kernel applies the gather+scale, data-parallel over the batch
dim across 8 NeuronCores.

Per core: read its [B/8, C, H*W] batch shard in 128-channel tiles (HWDGE
sync ring — deals descriptors evenly over the 16 SDMA engines), scale each
outlier channel by 1/T on DVE (per-partition tensor_scalar), and write each
scaled channel tile out T times via SWDGE (gpsimd) strided stores — measured
the best-balanced engine distribution for SBUF->HBM on this shape.
"""

import numpy as np

THRESHOLD = 8.0
B, C, H, W = 16, 512, 56, 56
HW = H * W
N_CORES = 8
BSH = B // N_CORES  # batches per core
P = 128  # SBUF partitions

PROFILE = False  # test harness sets True to collect exec_time_ns
STORE_BROADCAST = False  # one broadcast-src store vs T strided stores
STORE_ENGINE = "gpsimd"  # which engine issues store DMAs: scalar|sync|gpsimd
LOAD_ENGINE = "sync"  # which engine issues load DMAs
STORE_SPLIT = 1  # split each store DMA into N column chunks
STORE_PSPLIT = 1  # split each store DMA into N partition chunks
COL_CHUNKS = 1  # process each channel tile in N independent column chunks
BUFS = 6  # io tile pool depth
RAW = False  # raw bacc pipeline instead of TileContext
LAST_EXEC_NS = None
LAST_RESULTS = None

_build_cache = {}


def _plan(x):
    max_vals = np.max(np.abs(x), axis=(0, 2, 3))
    outlier = np.nonzero(max_vals > THRESHOLD)[0]
    T = np.ceil(max_vals[outlier] / THRESHOLD).astype(np.int64)
    return outlier, T


def _tile_runs(outlier, T, starts, i, n_out):
    """Contiguous-source load runs and equal-T store runs for channel tile i."""
    lo, hi = i * P, min((i + 1) * P, n_out)
    chans = outlier[lo:hi]
    loads = []  # (dst_partition, src_channel, run_len)
    rs = 0
    for k in range(1, len(chans) + 1):
        if k == len(chans) or chans[k] != chans[k - 1] + 1:
            loads.append((rs, int(chans[rs]), k - rs))
            rs = k
    stores = []  # (partition0, run_len, T, out_channel_start)
    ts, st = T[lo:hi], starts[lo:hi]
    rs = 0
    for k in range(1, len(chans) + 1):
        if k == len(chans) or ts[k] != ts[rs]:
            stores.append((rs, k - rs, int(ts[rs]), int(st[rs])))
            rs = k
    return loads, stores, hi - lo


def _build(outlier, T):
    import concourse.bacc as bacc
    import concourse.mybir as mybir
    import concourse.tile as tile

    f32 = mybir.dt.float32
    n_out = len(outlier)
    n_tiles = (n_out + P - 1) // P
    starts = np.concatenate([[0], np.cumsum(T)[:-1]]).astype(np.int64)
    R = int(T.sum())

    nc = bacc.Bacc(None)
    x_d = nc.declare_dram_parameter("x", [BSH * C, HW], f32, isOutput=False)
    s_d = nc.declare_dram_parameter("s", [P, n_tiles], f32, isOutput=False)
    y_d = nc.declare_dram_parameter("y", [BSH * R, HW], f32, isOutput=True)

    ccsz = (HW + COL_CHUNKS - 1) // COL_CHUNKS
    col_chunks = []
    for s in range(COL_CHUNKS):
        g0 = s * ccsz
        g1 = min(HW, g0 + ccsz)
        if g0 < g1:
            col_chunks.append((g0, g1))

    with tile.TileContext(nc) as tc:
        with (
            tc.tile_pool(name="sc", bufs=1) as scp,
            tc.tile_pool(name="io", bufs=BUFS) as iop,
        ):
            sc = scp.tile([P, n_tiles], f32)
            nc.sync.dma_start(sc[:], s_d[:])
            for b in range(BSH):
                for i in range(n_tiles):
                    loads, stores, p = _tile_runs(outlier, T, starts, i, n_out)
                    st_eng = getattr(nc, STORE_ENGINE)
                    ld_eng = getattr(nc, LOAD_ENGINE)
                    for g0, g1 in col_chunks:
                        gw = g1 - g0
                        t = iop.tile([P, ccsz], f32, tag="io")
                        for dp, src_c, ln in loads:
                            ld_eng.dma_start(
                                t[dp : dp + ln, :gw],
                                x_d[b * C + src_c : b * C + src_c + ln, g0:g1],
                            )
                        nc.vector.tensor_scalar_mul(
                            t[:p, :gw], t[:p, :gw], sc[:p, i : i + 1]
                        )
                        for p0, ln, tt, os_ in stores:
                            grp = y_d[
                                b * R + os_ : b * R + os_ + ln * tt, :
                            ].rearrange("(l t) f -> l t f", t=tt)
                            if STORE_BROADCAST:
                                src = t[p0 : p0 + ln, :gw].unsqueeze(1).broadcast_to(
                                    [ln, tt, gw]
                                )
                                st_eng.dma_start(grp[:, :, g0:g1], src)
                            else:
                                ssz = (gw + STORE_SPLIT - 1) // STORE_SPLIT
                                psz = (ln + STORE_PSPLIT - 1) // STORE_PSPLIT
                                for trep in range(tt):
                                    for q in range(STORE_PSPLIT):
                                        q0 = q * psz
                                        q1 = min(ln, q0 + psz)
                                        if q0 >= q1:
                                            continue
                                        for s in range(STORE_SPLIT):
                                            sc0 = s * ssz
                                            sc1 = min(gw, sc0 + ssz)
                                            if sc0 >= sc1:
                                                continue
                                            st_eng.dma_start(
                                                grp[q0:q1, trep, g0 + sc0 : g0 + sc1],
                                                t[p0 + q0 : p0 + q1, sc0:sc1],
                                            )
    nc.finalize()  # Bacc: split multi-wait instructions, allocate registers
    return nc


def _build_raw(outlier, T):
    """Raw bacc pipeline (no TileContext): sync loads -> DVE scale -> gpsimd
    stores, with per-slot semaphores for flow control.  Saves the Tile
    scope/drain scaffolding (~15-20us)."""
    import concourse.bacc as bacc
    import concourse.mybir as mybir

    f32 = mybir.dt.float32
    n_out = len(outlier)
    n_tiles = (n_out + P - 1) // P
    starts = np.concatenate([[0], np.cumsum(T)[:-1]]).astype(np.int64)
    R = int(T.sum())

    nc = bacc.Bacc(None)
    x_d = nc.declare_dram_parameter("x", [BSH * C, HW], f32, isOutput=False)
    s_d = nc.declare_dram_parameter("s", [P, n_tiles], f32, isOutput=False)
    y_d = nc.declare_dram_parameter("y", [BSH * R, HW], f32, isOutput=True)

    units = []  # (b, i, loads, stores, p)
    for b in range(BSH):
        for i in range(n_tiles):
            loads, stores, p = _tile_runs(outlier, T, starts, i, n_out)
            nst = sum(tt for (_, _, tt, _) in stores)
            units.append((b, i, loads, stores, p, len(loads), nst))
    n_units = len(units)
    nbufs = min(BUFS, n_units)

    # cumulative per-slot load/store DMA counts (for wait thresholds)
    slot_ld_cum = [0] * nbufs
    slot_st_cum = [0] * nbufs
    ld_target = [0] * n_units  # ld_sem[slot] value once unit k's loads done
    st_target = [0] * n_units  # st_sem[slot] value once unit k's stores done

    with (
        nc.sbuf_tensor([P, nbufs * HW], f32) as buf,
        nc.sbuf_tensor([P, n_tiles], f32) as sc,
        nc.Block() as block,
    ):
        ld_sems = [nc.alloc_semaphore(f"ld{s}") for s in range(nbufs)]
        st_sems = [nc.alloc_semaphore(f"st{s}") for s in range(nbufs)]
        ve_sem = nc.alloc_semaphore("ve")
        sc_sem = nc.alloc_semaphore("scld")

        for k, (b, i, loads, stores, p, nld, nst) in enumerate(units):
            s = k % nbufs
            slot_ld_cum[s] += nld
            slot_st_cum[s] += nst
            ld_target[k] = slot_ld_cum[s]
            st_target[k] = slot_st_cum[s]

        @block.sync
        def _(sync):
            sync.dma_start(sc[:, :], s_d[:, :]).then_inc(sc_sem, 16)
            for k, (b, i, loads, stores, p, nld, nst) in enumerate(units):
                s = k % nbufs
                tile = buf[:, s * HW : (s + 1) * HW]
                if k >= nbufs:
                    sync.wait_ge(st_sems[s], 16 * st_target[k - nbufs])
                for dp, src_c, ln in loads:
                    sync.dma_start(
                        tile[dp : dp + ln, :],
                        x_d[b * C + src_c : b * C + src_c + ln, :],
                    ).then_inc(ld_sems[s], 16)

        @block.vector
        def _(vector):
            vector.wait_ge(sc_sem, 16)
            for k, (b, i, loads, stores, p, nld, nst) in enumerate(units):
                s = k % nbufs
                tile = buf[:, s * HW : (s + 1) * HW]
                vector.wait_ge(ld_sems[s], 16 * ld_target[k])
                nc.vector.tensor_scalar_mul(
                    tile[:p, :], tile[:p, :], sc[:p, i : i + 1]
                ).then_inc(ve_sem, 1)

        @block.gpsimd
        def _(gpsimd):
            for k, (b, i, loads, stores, p, nld, nst) in enumerate(units):
                s = k % nbufs
                tile = buf[:, s * HW : (s + 1) * HW]
                gpsimd.wait_ge(ve_sem, k + 1)
                for p0, ln, tt, os_ in stores:
                    grp = y_d[
                        b * R + os_ : b * R + os_ + ln * tt, :
                    ].rearrange("(l t) f -> l t f", t=tt)
                    for trep in range(tt):
                        gpsimd.dma_start(
                            grp[:, trep, :], tile[p0 : p0 + ln, :]
                        ).then_inc(st_sems[s], 16)
            for s in range(nbufs):
                gpsimd.wait_ge(st_sems[s], 16 * slot_st_cum[s])

    nc.finalize()
    return nc


def kernel(x):
    global LAST_EXEC_NS, LAST_RESULTS
    from concourse.bass_utils import run_bass_kernel_spmd

    x = np.ascontiguousarray(np.asarray(x), dtype=np.float32)
    assert x.shape == (B, C, H, W), x.shape

    outlier, T = _plan(x)
    R = int(T.sum())
    if R == 0:
        return (
            np.zeros((B, 0, H, W), np.float32),
            outlier.astype(np.int32),
        )

    key = (outlier.tobytes(), T.tobytes(), STORE_BROADCAST, STORE_ENGINE, LOAD_ENGINE, STORE_SPLIT, STORE_PSPLIT, COL_CHUNKS, BUFS, RAW)
    if key not in _build_cache:
        _build_cache[key] = (_build_raw if RAW else _build)(outlier, T)
    nc = _build_cache[key]

    n_out = len(outlier)
    n_tiles = (n_out + P - 1) // P
    inv_T = (1.0 / T).astype(np.float32)
    svec = np.ones((P, n_tiles), np.float32)
    for j in range(n_out):
        svec[j % P, j // P] = inv_T[j]

    xs = x.reshape(B, C, HW)
    in_maps = [
        {"x": np.ascontiguousarray(xs[c * BSH : (c + 1) * BSH].reshape(BSH * C, HW)),
         "s": svec}
        for c in range(N_CORES)
    ]
    res = run_bass_kernel_spmd(
        nc, in_maps, core_ids=list(range(N_CORES)), trace=PROFILE
    )
    LAST_EXEC_NS = res.exec_time_ns
    LAST_RESULTS = res
    out = np.concatenate(
        [r["y"].reshape(BSH, R, H, W) for r in res.results], axis=0
    )
    return out, outlier.astype(np.int32)
